# revision 44
# baseline (speedup 1.0000x reference)
"""Trainium2 Bass kernel for nn_DepatchSampling (v3).

Strategy (hardcoded for B=32, C=64, L=4096, PS=16, STRIDE=8, PC=511, HID=64):

 - Pure data parallelism: batch dim (32) sharded over 8 cores, 4 batches each.
 - Per core, 256 (b,c) rows in 2 chunks of 128 rows (one row per partition).
 - X rows are cast to bf16 (xh) and block-transposed L-major via the DMA
   xbar (dma_start_transpose) -> xth; conv1 runs on the PE in bf16
   (1 cycle/row vs 4 for fp32): per patch-pair t one K=128 matmul with a
   pre-packed W1 variant (rho = 16*(t%8); the rho==112 pair splits into two
   accumulating matmuls across the block boundary).
 - gelu(+b1) on ACT (the only ACT work; ~64us is the design ceiling);
   conv2 uses h as the (free-in-the-cost-model) stationary operand and a
   packed [128,4] W2 moving operand -> offsets [(b,c), (t,4)] in PSUM.
 - Decode exploits that anchors cancel: interior patches need only
       ds  = relu(ds_raw + b2[1] + 7.5)
       A   = dx_raw + (b2[0] + 8.5) - ds          (= lo' - 8p + 1)
       G   = ds*(2/15) - 1                        (= (hi'-lo')/15 - 1)
   Clipping only activates for p=0 (lo) and p=510 (hi); those two columns
   are recomputed exactly. p=511 is computed but discarded.
 - Interpolation per element (u = A + G*s in [0,2], b = 8p+s-1):
       out = X[b] + min(u,1)*D1[b] + relu(u-1)*D1[b+1]
           = X[b] - min(u,1)*D2[b+1] + u*D1[b+1]
   exact piecewise-linear interpolation (extrapolates consistently under
   bf16 rounding of u). All X/D1/D2 accesses are static strided views.
   Interp runs in bf16 (DVE 2x/4x perf modes); the final fp32 add runs on
   GPSIMD (DVE for the tail quarter).
 - DMA program: X loads + xbar transposes for BOTH chunks are issued on the
   SP sequencer before any OUT store (in-order seq waits would otherwise
   stall chunk 1's loads behind chunk 0's interp); consts go via the ACT
   sequencer; outputs are stored per quarter-chunk to shorten the tail.
"""

import numpy as np
import ml_dtypes

import concourse.bass as bass
import concourse.bacc as bacc
import concourse.mybir as mybir
from concourse.tile import TileContext
from concourse.bass_utils import run_bass_kernel_spmd

F32 = mybir.dt.float32
BF16 = mybir.dt.bfloat16
AF = mybir.ActivationFunctionType
OP = mybir.AluOpType

# Problem constants
B, C, L = 32, 64, 4096
PS, STRIDE, PC, HID = 16, 8, 511, 64
NCORES = 8
BPC = B // NCORES            # batches per core
ROWS = BPC * C               # 256 (b,c) rows per core
NCHUNK = 2
NT = 256                     # patch-pair index t: p = 2t, 2t+1
XW = 4112                    # xh/d1h/d2h padded width
GRP = 12                     # conv1/gelu group size (in t); 12*128 f32 = 3 PSUM banks

_CACHE = {}


def _consts(W1, b1, W2, b2):
    """Host-side packing of weights (bf16) and scalars."""
    W1 = np.asarray(W1, np.float32)
    b1 = np.asarray(b1, np.float32)
    W2 = np.asarray(W2, np.float32)
    b2 = np.asarray(b2, np.float32)

    bf = ml_dtypes.bfloat16
    tens = {}
    w1all = np.zeros((128, 9 * 128), np.float32)
    for k, rho in enumerate(range(0, 112, 16)):
        w1all[rho:rho + 16, 128 * k:128 * k + 64] = W1.T
        w1all[rho + 8:rho + 24, 128 * k + 64:128 * k + 128] = W1.T
    w1all[112:128, 896:960] = W1.T          # W1SA at block 7
    w1all[120:128, 960:1024] = W1.T[0:8]
    w1all[0:8, 1088:1152] = W1.T[8:16]      # W1SB at block 8
    tens["W1ALL"] = w1all.astype(bf)

    w2p = np.zeros((128, 4), np.float32)
    w2p[0:64, 0] = W2[0]
    w2p[0:64, 1] = W2[1]
    w2p[64:128, 2] = W2[0]
    w2p[64:128, 3] = W2[1]
    tens["W2P"] = w2p.astype(bf)
    tens["B1P"] = np.concatenate([b1, b1]).reshape(128, 1).astype(np.float32)

    scal = {
        "c_ds": float(np.float32(b2[1]) + np.float32(7.5)),
        "a_sc": float(np.float32(b2[0]) + np.float32(8.5)),
        "b20": float(np.float32(b2[0])),
    }
    return tens, scal


def _ap(tile_ap, col_off, dims):
    """Strided view of a 2D [128, F] tile: dims = [[step, count], ...]."""
    pstep = tile_ap.ap[0][0]
    npart = tile_ap.ap[0][1]
    return bass.AP(tile_ap.tensor, tile_ap.offset + col_off,
                   [[pstep, npart]] + [list(d) for d in dims])


CONST_SHAPES = {"W2P": (128, 4), "B1P": (128, 1), "W1ALL": (128, 9 * 128)}


def build(scal):
    nc = bacc.Bacc("TRN2", target_bir_lowering=False, debug=False)

    XS = nc.dram_tensor("XS", [ROWS, L], F32, kind="ExternalInput")
    OUT = nc.dram_tensor("OUT", [BPC, C, PC, PS], F32, kind="ExternalOutput")
    cdram = {}
    for k, s in CONST_SHAPES.items():
        dt = F32 if k == "B1P" else BF16
        cdram[k] = nc.dram_tensor(k, list(s), dt, kind="ExternalInput")

    c_ds, a_sc, b20 = scal["c_ds"], scal["a_sc"], scal["b20"]

    groups = [(0, 4)]
    t0 = 4
    while t0 < NT:
        groups.append((t0, min(GRP, NT - t0)))
        t0 += groups[-1][1]

    with TileContext(nc) as tc:
        with tc.tile_pool(name="consts", bufs=1) as cpool, \
             tc.tile_pool(name="xq", bufs=4) as xqpool, \
             tc.tile_pool(name="stat", bufs=2) as spool, \
             tc.tile_pool(name="work", bufs=2) as wpool, \
             tc.tile_pool(name="psum", bufs=1, space="PSUM") as ppool:

            csb = {}
            for k, s in CONST_SHAPES.items():
                dt = F32 if k == "B1P" else BF16
                t = cpool.tile([s[0], s[1]], dt, tag=f"c_{k}")
                csb[k] = t

            # ---- Phase A: load/cast/transpose/diffs for BOTH chunks ----
            # All 8 X loads issue back-to-back on the SP seq; the xbar
            # transposes for chunk 0 ride the ACT seq (fires right after the
            # consts), chunk 1's go on SP after the loads.  This keeps every
            # in-order sequencer free of waits on late producers.
            for k in ("W1ALL", "W2P", "B1P"):
                nc.scalar.dma_start(csb[k][:, :], cdram[k][:, :])

            ch = []
            tiles = []
            for chunk in range(NCHUNK):
                r0 = chunk * 128
                xh = spool.tile([128, XW], BF16, tag="xh",
                                name=f"xh{chunk}")
                nc.vector.memset(xh[:, 0:1], 0.0)
                nc.vector.memset(xh[:, 1 + L:XW], 0.0)
                xth = spool.tile([128, L], BF16, tag="xth",
                                 name=f"xth{chunk}")
                tiles.append((r0, xh, xth))
                ch.append((r0, xh, xth))

            # chunk 0 in 4 quarters (low latency to first conv), chunk 1 in
            # 2 halves (fewer seq round-trips; its deadline is much later)
    
            pieces = [(0, 0, 512), (0, 512, 1536), (0, 2048, 1024),
                      (0, 3072, 1024)] + \
                     [(1, 2048 * hh, 2048) for hh in range(2)]

            def emit_load(i):
                chunk, c0, w = pieces[i]
                r0, xh, xth = tiles[chunk]
                xq = xqpool.tile([128, 2048], F32, tag="xq",
                                 name=f"xq{chunk}_{c0}")
                nc.sync.dma_start(xq[:, :w], XS[r0:r0 + 128, c0:c0 + w])
                if i == 0:
                    nc.gpsimd.tensor_copy(xh[:, 1:1 + w], xq[:, 0:w])
                elif i == 1:
                    nc.gpsimd.tensor_copy(xh[:, 1 + c0:1 + c0 + 768],
                                          xq[:, 0:768])
                    nc.gpsimd.tensor_copy(xh[:, 1 + c0 + 768:1 + c0 + w],
                                          xq[:, 768:w])
                else:
                    nc.gpsimd.tensor_copy(xh[:, 1 + c0:1 + c0 + w], xq[:, :w])

            def emit_dmat(i):
                chunk, c0, w = pieces[i]
                r0, xh, xth = tiles[chunk]
                xtv = bass.AP(xth[:, :].tensor, xth[:, :].offset + c0,
                              [list(xth[:, :].ap[0]), [128, w // 128],
                               [1, 128]])
                nc.sync.dma_start_transpose(xtv, xh[:, 1 + c0:1 + c0 + w])

            # SP seq: chunk-0 loads+transposes first (minimal HWDGE
            # contention for the conv-critical pieces); chunk 1 trails
            for i in range(4):
                emit_load(i)
            emit_dmat(0)
            emit_dmat(1)
            emit_dmat(2)
            emit_dmat(3)
            emit_load(4)
            emit_dmat(4)
            emit_load(5)
            emit_dmat(5)
            for i in range(NCHUNK):
                r0, xh, xth = ch[i]
                # d1h[:, j] = D1[j-1] = X[j]-X[j-1]; d2h[:, j] = D2[j]
                d1h = spool.tile([128, XW], BF16, tag="d1h",
                                 name=f"d1h{i}")
                nc.vector.tensor_sub(d1h[:, 0:L + 3],
                                     xh[:, 1:L + 4], xh[:, 0:L + 3])
                nc.vector.memset(d1h[:, L + 3:XW], 0.0)
                d2h = spool.tile([128, XW], BF16, tag="d2h",
                                 name=f"d2h{i}")
                nc.vector.tensor_sub(d2h[:, 0:L + 2],
                                     d1h[:, 1:L + 3], d1h[:, 0:L + 2])
                nc.vector.memset(d2h[:, L + 2:XW], 0.0)
                ch[i] = (r0, xh, xth, d1h, d2h)

            # ---- Phase B: conv -> decode -> interp -> store, per chunk ----
            for chunk in range(NCHUNK):
                r0, xh, xth, d1h, d2h = ch[chunk]
                Ac = spool.tile([128, 512], BF16, tag="Ac", name=f"Ac{chunk}")
                Gc = spool.tile([128, 512], BF16, tag="Gc", name=f"Gc{chunk}")

                offq = [None] * 4
                for (tg0, ntg) in groups:
                    pt = ppool.tile([128, GRP * 128], F32, tag="pt", bufs=2)
                    for j in range(ntg):
                        t = tg0 + j
                        blkA, rho = divmod(16 * t, 128)
                        dst = pt[:, 128 * j:128 * (j + 1)]
                        W1A = csb["W1ALL"]
                        if rho <= 96:
                            k = rho // 16
                            nc.tensor.matmul(
                                dst, W1A[:, 128 * k:128 * (k + 1)],
                                xth[:, 128 * blkA:128 * (blkA + 1)],
                                start=True, stop=True)
                        elif t == NT - 1:
                            nc.tensor.matmul(
                                dst, W1A[64:128, 896:1024],
                                xth[64:128, 128 * blkA:128 * (blkA + 1)],
                                start=True, stop=True)
                        else:
                            nc.tensor.matmul(
                                dst, W1A[64:128, 896:1024],
                                xth[64:128, 128 * blkA:128 * (blkA + 1)],
                                start=True, stop=False)
                            nc.tensor.matmul(
                                dst, W1A[0:8, 1024:1152],
                                xth[0:8, 128 * (blkA + 1):128 * (blkA + 2)],
                                start=False, stop=True)
                    hsb = wpool.tile([128, GRP * 128], BF16, tag="hsb", bufs=3)
                    nc.scalar.activation(hsb[:, :128 * ntg], pt[:, :128 * ntg],
                                         AF.Gelu, bias=csb["B1P"][:, 0:1],
                                         scale=1.0)
                    for j in range(ntg):
                        t = tg0 + j
                        qi = t // 64
                        if t % 64 == 0:
                            offq[qi] = ppool.tile([128, 256], F32,
                                                  tag="offpt", bufs=2,
                                                  name=f"off{chunk}_{qi}")
                        nc.tensor.matmul(
                            offq[qi][:, 4 * (t - 64 * qi):4 * (t - 64 * qi) + 4],
                            hsb[:, 128 * j:128 * (j + 1)], csb["W2P"][:, :],
                            start=True, stop=True)
                        if t % 64 == 63:
                            lastq = (chunk == NCHUNK - 1 and qi == 3)
                            with tc.high_priority(
                                    offset=None if lastq else 200):
                                _decode(nc, wpool, offq[qi], qi, 0, 128,
                                        Ac, Gc, c_ds, a_sc, b20)

                # interp per quarter (last chunk: final quarter as two
                # eighths so the post-gelu tail is a 64-patch chain)
                ipieces = [(0, 128), (128, 128), (256, 128), (384, 128)]
                for p0, npp in ipieces:
                    nf = 16 * npp
                    ut = spool.tile([128, 2048], BF16, tag="u",
                                    name=f"u{chunk}_{p0}")
                    lastu = (chunk == NCHUNK - 1 and p0 == 384)
                    with tc.high_priority(offset=400 if lastu else 0):
                        for s in range(16):
                            uv = _ap(ut[:, :], s, [[16, npp]])
                            nc.vector.scalar_tensor_tensor(
                                uv, Gc[:, p0:p0 + npp], float(s),
                                Ac[:, p0:p0 + npp], OP.mult, OP.add)
                    u = ut[:, :]
                    d1p = _ap(d1h[:, :], 1 + 8 * p0, [[8, npp], [1, 16]])
                    d2p = _ap(d2h[:, :], 8 * p0, [[8, npp], [1, 16]])
                    x_v = _ap(xh[:, :], 8 * p0, [[8, npp], [1, 16]])
                    p2 = wpool.tile([128, 2048], BF16, tag="p2", bufs=2,
                                    name=f"p2_{chunk}_{p0}")
                    nc.vector.tensor_mul(p2[:, :nf], u[:, :nf], d1p)
                    vt = wpool.tile([128, 2048], BF16, tag="vt", bufs=2,
                                    name=f"vt_{chunk}_{p0}")
                    nc.vector.tensor_scalar(vt[:, :nf], u[:, :nf], 1.0,
                                            1.0, OP.min, OP.mult)
                    last = (chunk == NCHUNK - 1 and p0 == 384)
                    p1 = wpool.tile([128, 2048], BF16, tag="p1", bufs=2,
                                    name=f"p1_{chunk}_{p0}")
                    if last:
                        for hh in range(2):
                            cl = slice(1024 * hh, 1024 * (hh + 1))
                            d2ph = _ap(d2h[:, :], 8 * (p0 + 64 * hh),
                                       [[8, 64], [1, 16]])
                            x_vh = _ap(xh[:, :], 8 * (p0 + 64 * hh),
                                       [[8, 64], [1, 16]])
                            nc.vector.tensor_mul(p1[:, cl], vt[:, cl], d2ph)
                            nc.vector.tensor_sub(p1[:, cl], x_vh, p1[:, cl])
                    else:
                        nc.vector.tensor_mul(p1[:, :nf], vt[:, :nf], d2p)
                        nc.vector.tensor_sub(p1[:, :nf], x_v, p1[:, :nf])
                    nout = nf if p0 + npp < 512 else nf - 16
                    outf = wpool.tile([128, 2048], F32, tag="outf", bufs=3,
                                      name=f"outf{chunk}_{p0}")
                    if last:
                        for piece in range(4):
                            c0 = (nout // 4) * piece
                            c1 = nout if piece == 3 else nout // 4 * (piece + 1)
                            nc.vector.tensor_add(outf[:, c0:c1],
                                                 p1[:, c0:c1], p2[:, c0:c1])
                            oap = bass.AP(OUT[:].tensor,
                                          r0 * PC * PS + 16 * p0 + c0,
                                          [[PC * PS, 128], [1, c1 - c0]])
                            nc.sync.dma_start(oap, outf[:, c0:c1])
                    else:
                        nc.gpsimd.tensor_add(outf[:, :nout], p1[:, :nout],
                                             p2[:, :nout])
                        oap = bass.AP(OUT[:].tensor, r0 * PC * PS + 16 * p0,
                                      [[PC * PS, 128], [1, nout]])
                        nc.sync.dma_start(oap, outf[:, :nout])
    nc.finalize()
    return nc


def _decode(nc, wpool, offt, qi, j0, np_, Ac, Gc, c_ds, a_sc, b20):
    """Decode patches [128*qi + j0, +np_) from offq tile cols [2*j0...)."""
    g0 = 128 * qi + j0
    dxv = _ap(offt[:, :], 2 * j0, [[2, np_]])
    dsv = _ap(offt[:, :], 2 * j0 + 1, [[2, np_]])
    dsb = wpool.tile([128, 128], F32, tag="dsb", bufs=2)
    nc.vector.tensor_scalar(dsb[:, :np_], dsv, c_ds, 0.0, OP.add, OP.max)
    nc.vector.scalar_tensor_tensor(Ac[:, g0:g0 + np_], dxv, a_sc,
                                   dsb[:, :np_], OP.add, OP.subtract)
    nc.gpsimd.tensor_scalar(Gc[:, g0:g0 + np_], dsb[:, :np_], 2.0 / 15.0,
                            -1.0, OP.mult, OP.add)

    ft = wpool.tile([128, 8], F32, tag="ft", bufs=2)
    if g0 == 0:
        # p = 0: lo clips at 0.  lo_u = dx' + 7.5 - ds; lo' = max(lo_u, 0)
        dx0 = offt[:, 0:1]
        ds0 = dsb[:, 0:1]
        nc.vector.scalar_tensor_tensor(ft[:, 0:1], dx0, b20 + 7.5, ds0,
                                       OP.add, OP.subtract)        # lo_u
        nc.vector.tensor_scalar(ft[:, 1:2], ft[:, 0:1], 0.0, 1.0,
                                OP.max, OP.mult)                    # lo'
        nc.vector.tensor_scalar(Ac[:, 0:1], ft[:, 1:2], 1.0, 1.0,
                                OP.add, OP.mult)
        nc.vector.scalar_tensor_tensor(ft[:, 2:3], dx0, b20 + 7.5, ds0,
                                       OP.add, OP.add)              # hi_u
        nc.vector.tensor_sub(ft[:, 3:4], ft[:, 2:3], ft[:, 1:2])
        nc.vector.tensor_scalar(Gc[:, 0:1], ft[:, 3:4], 1.0 / 15.0, -1.0,
                                OP.mult, OP.add)
    if g0 <= 510 < g0 + np_:
        # p = 510 (t=255 even patch): hi clips at 4095.
        jc = 510 - 128 * qi                  # tile-local patch index
        dxc = offt[:, 2 * jc:2 * jc + 1]
        dsc = dsb[:, 510 - g0:510 - g0 + 1]
        nc.vector.scalar_tensor_tensor(ft[:, 4:5], dxc, b20 + 4087.5, dsc,
                                       OP.add, OP.subtract)         # lo_u
        nc.vector.scalar_tensor_tensor(ft[:, 5:6], dxc, b20 + 4087.5, dsc,
                                       OP.add, OP.add)              # hi_u
        nc.vector.tensor_scalar(ft[:, 6:7], ft[:, 5:6], 4095.0, 1.0,
                                OP.min, OP.mult)                    # hi'
        nc.vector.tensor_sub(ft[:, 7:8], ft[:, 6:7], ft[:, 4:5])
        nc.vector.tensor_scalar(Gc[:, 510:511], ft[:, 7:8], 1.0 / 15.0,
                                -1.0, OP.mult, OP.add)


def kernel(X, W1, b1, W2, b2):
    X = np.ascontiguousarray(np.asarray(X, np.float32))
    tens, scal = _consts(W1, b1, W2, b2)
    key = tuple(sorted(scal.items()))
    if _CACHE.get("key") != key:
        _CACHE["nc"] = build(scal)
        _CACHE["key"] = key
    nc = _CACHE["nc"]

    in_maps = []
    for i in range(NCORES):
        m = {"XS": X[BPC * i:BPC * (i + 1)].reshape(ROWS, L)}
        m.update(tens)
        in_maps.append(m)

    res = run_bass_kernel_spmd(nc, in_maps, core_ids=list(range(NCORES)))
    out = np.concatenate([res.results[i]["OUT"] for i in range(NCORES)], axis=0)
    return out
